# revision 3
# baseline (speedup 1.0000x reference)
"""MoE gate (DeepSeek-style MoEGate) for Trainium2, 8 NeuronCores.

Problem: hidden_states [8, 4096, 1024] f32, router weight [64, 1024] f32.
  logits = tokens @ W.T ; scores = softmax(logits)
  top-8 indices/weights (renormalized), expert counts, seq-aux loss.

Sharding: data-parallel over the batch axis — core b handles batch row b
(4096 tokens). The router weight is replicated. Expert counts and the
aux-loss terms are computed per-core on device ([64] score sums + [64]
counts) and combined on the host (the "all-reduce" of the hint).

Per-core device pipeline (tokens on partitions, 128 per tile):
  DMA x [128,1024]-tiles -> PE transpose (128x128 blocks) -> PSUM -> copy
  to SBUF (x^T) -> PE matmul vs W^T chunks accumulating logits [128,64]
  -> max8/max_index (top-8 of 64 per partition) -> exp/sums on ACT/DVE
  -> renormalized top-8 weights; scores and top-8 mask are reduced over
  tokens with a ones-vector PE matmul into a persistent PSUM accumulator.
"""

import sys

if "/opt/trn_rl_repo" not in sys.path:
    sys.path.insert(0, "/opt/trn_rl_repo")

import numpy as np

import concourse.bass as bass
import concourse.mybir as mybir
import concourse.tile as tile
from concourse.bass import ds
from concourse.bass_utils import run_bass_kernel_spmd
from concourse.masks import make_identity

F32 = mybir.dt.float32
U32 = mybir.dt.uint32

B, S, H = 8, 4096, 1024
E, K = 64, 8
ALPHA = 0.01
N_CORES = 8

P = 128          # partitions / tokens per tile
HC = H // P      # 8 h-chunks of 128
SUP = 8          # token-tiles per super-tile
N_SUP = S // (SUP * P)   # 4 super-tiles per core


def _split_multi_waits(nc: bass.Bass, limit: int = 1):
    """This container's walrus build rejects any instruction carrying more
    than one semaphore wait (setupSyncWait: 'Too many sync wait commands'),
    while Tile freely attaches several. Post-pass: hoist excess waits onto
    single-wait NoOps inserted just before the instruction on the same
    engine (the engine's program-order queue preserves semantics)."""
    n = 0
    for f in nc.m.functions:
        for bb in f.blocks:
            insts = bb.instructions
            out = []
            changed = False
            for inst in insts:
                si = inst.sync_info
                if si is not None and len(si.on_wait) > limit:
                    waits = list(si.on_wait)
                    for w in waits[:-limit]:
                        nop = mybir.InstNoOp(
                            name=f"I-waitsplit-{n}",
                            sync_info=mybir.SyncInfo(on_update=[], on_wait=[w]),
                            bass_nofuse=True,
                            engine=inst.engine,
                        )
                        n += 1
                        out.append(nop)
                    si.on_wait = waits[-limit:]
                    changed = True
                out.append(inst)
            if changed:
                bb.instructions = out
    return n


def build_nc(repeat: int = 1) -> bass.Bass:
    """Build the per-core Bass program. repeat>1 replays the main token loop
    (for timing only; outputs are then garbage for the repeated passes)."""
    nc = bass.Bass()

    x_ext = nc.declare_dram_parameter("x", [S, H], F32, isOutput=False)
    w_ext = nc.declare_dram_parameter("w", [E, H], F32, isOutput=False)
    idx_ext = nc.declare_dram_parameter("idx", [S, K], U32, isOutput=True)
    wts_ext = nc.declare_dram_parameter("wts", [S, K], F32, isOutput=True)
    # stats rows 0:64  = sum over this core's tokens of softmax scores
    #       rows 64:128 = top-8 membership counts per expert
    stats_ext = nc.declare_dram_parameter("stats", [2 * E, 1], F32, isOutput=True)

    with tile.TileContext(nc) as tc:
        with tc.tile_pool(name="const", bufs=1) as const_pool:
            identity = const_pool.tile([P, P], F32)
            make_identity(nc, identity[:])
            ones = const_pool.tile([P, 1], F32)
            nc.vector.memset(ones[:], 1.0)

            # W^T chunks: wt[p, c, e] = W[e, c*128 + p]
            wt = const_pool.tile([P, HC, E], F32)
            w_sb = const_pool.tile([E, H], F32)
            nc.sync.dma_start(w_sb[:], w_ext[:])
            with tc.tile_pool(name="wt_psum", bufs=2, space="PSUM") as wt_psum_pool:
                for c in range(HC):
                    wt_ps = wt_psum_pool.tile([P, E], F32)
                    nc.tensor.transpose(
                        wt_ps[:], w_sb[:, ds(c * P, P)], identity[0:E, 0:E]
                    )
                    nc.vector.tensor_copy(wt[:, c, :], wt_ps[:])

            with (
                tc.tile_pool(name="x", bufs=2) as x_pool,
                tc.tile_pool(name="xT", bufs=2) as xT_pool,
                tc.tile_pool(name="lg", bufs=2) as lg_pool,
                tc.tile_pool(name="cb", bufs=2) as cb_pool,
                tc.tile_pool(name="sm", bufs=2) as sm_pool,
                tc.tile_pool(name="xT_psum", bufs=4, space="PSUM") as xT_psum_pool,
                tc.tile_pool(name="lg_psum", bufs=2, space="PSUM") as lg_psum_pool,
                tc.tile_pool(name="st_psum", bufs=1, space="PSUM") as st_psum_pool,
            ):
                stats_psum = st_psum_pool.tile([2 * E, 1], F32)

                for r in range(repeat):
                    for st in range(N_SUP):
                        t0 = st * SUP * P
                        x_t = x_pool.tile([P, SUP, H], F32)
                        for half in range(2):
                            src = x_ext[ds(t0 + half * 512, 512), :].rearrange(
                                "(s p) h -> p s h", p=P
                            )
                            nc.sync.dma_start(x_t[:, ds(half * 4, 4), :], src)

                        lg_psum = lg_psum_pool.tile([P, SUP, E], F32)
                        for j in range(SUP):
                            xT = xT_pool.tile([P, HC, P], F32)
                            for g in range(2):
                                xp = xT_psum_pool.tile([P, 4, P], F32)
                                for cc in range(4):
                                    c = g * 4 + cc
                                    nc.tensor.transpose(
                                        xp[:, cc, :],
                                        x_t[:, j, ds(c * P, P)],
                                        identity[:],
                                    )
                                if g == 0:
                                    nc.vector.tensor_copy(
                                        xT[:, ds(g * 4, 4), :], xp[:]
                                    )
                                else:
                                    nc.scalar.copy(xT[:, ds(g * 4, 4), :], xp[:])
                            for c in range(HC):
                                nc.tensor.matmul(
                                    lg_psum[:, j, :],
                                    xT[:, c, :],
                                    wt[:, c, :],
                                    start=(c == 0),
                                    stop=(c == HC - 1),
                                )

                        lg = lg_pool.tile([P, SUP, E], F32)
                        nc.scalar.copy(lg[:], lg_psum[:])

                        v8 = sm_pool.tile([P, SUP, K], F32)
                        i8 = sm_pool.tile([P, SUP, K], U32)
                        for j in range(SUP):
                            nc.vector.max(out=v8[:, j, :], in_=lg[:, j, :])
                            nc.vector.max_index(
                                out=i8[:, j, :], in_max=v8[:, j, :],
                                in_values=lg[:, j, :],
                            )

                        # combined lhsT for the stats matmul:
                        # cols 0:64 = softmax scores, cols 64:128 = top-8 mask
                        cb = cb_pool.tile([P, SUP, 2 * E], F32)
                        nc.scalar.activation(
                            out=cb[:, :, 0:E], in_=lg[:],
                            func=mybir.ActivationFunctionType.Exp,
                        )
                        zs = sm_pool.tile([P, SUP, 1], F32)
                        nc.vector.tensor_reduce(
                            out=zs[:], in_=cb[:, :, 0:E],
                            axis=mybir.AxisListType.X, op=mybir.AluOpType.add,
                        )
                        zr = sm_pool.tile([P, SUP, 1], F32)
                        nc.vector.reciprocal(zr[:], zs[:])
                        nc.vector.tensor_tensor(
                            out=cb[:, :, 0:E], in0=cb[:, :, 0:E],
                            in1=zr[:].to_broadcast([P, SUP, E]),
                            op=mybir.AluOpType.mult,
                        )
                        nc.vector.tensor_tensor(
                            out=cb[:, :, E : 2 * E], in0=lg[:],
                            in1=v8[:, :, K - 1 : K].to_broadcast([P, SUP, E]),
                            op=mybir.AluOpType.is_ge,
                        )
                        for j in range(SUP):
                            nc.tensor.matmul(
                                stats_psum[:],
                                cb[:, j, :],
                                ones[:],
                                start=(r == 0 and st == 0 and j == 0),
                                stop=(r == repeat - 1 and st == N_SUP - 1
                                      and j == SUP - 1),
                                skip_group_check=True,
                            )

                        # renormalized top-8 weights: exp(v)/sum8(exp(v))
                        t8 = sm_pool.tile([P, SUP, K], F32)
                        nc.scalar.activation(
                            out=t8[:], in_=v8[:],
                            func=mybir.ActivationFunctionType.Exp,
                        )
                        s8 = sm_pool.tile([P, SUP, 1], F32)
                        nc.vector.tensor_reduce(
                            out=s8[:], in_=t8[:],
                            axis=mybir.AxisListType.X, op=mybir.AluOpType.add,
                        )
                        s8r = sm_pool.tile([P, SUP, 1], F32)
                        nc.vector.reciprocal(s8r[:], s8[:])
                        wout = sm_pool.tile([P, SUP, K], F32)
                        nc.vector.tensor_tensor(
                            out=wout[:], in0=t8[:],
                            in1=s8r[:].to_broadcast([P, SUP, K]),
                            op=mybir.AluOpType.mult,
                        )

                        dst_i = idx_ext[ds(t0, SUP * P), :].rearrange(
                            "(s p) k -> p s k", p=P
                        )
                        nc.scalar.dma_start(dst_i, i8[:])
                        dst_w = wts_ext[ds(t0, SUP * P), :].rearrange(
                            "(s p) k -> p s k", p=P
                        )
                        nc.scalar.dma_start(dst_w, wout[:])

                stats_sb = sm_pool.tile([2 * E, 1], F32)
                nc.vector.tensor_copy(stats_sb[:], stats_psum[:])
                nc.scalar.dma_start(stats_ext[:], stats_sb[:])

    nsplit = _split_multi_waits(nc)
    return nc


_NC_CACHE = {}


def _get_nc(repeat: int = 1) -> bass.Bass:
    if repeat not in _NC_CACHE:
        _NC_CACHE[repeat] = build_nc(repeat)
    return _NC_CACHE[repeat]


def _combine(results):
    idx = np.concatenate(
        [r["idx"].astype(np.int32) for r in results], axis=0
    )
    wts = np.concatenate([r["wts"] for r in results], axis=0)
    score_sums = np.stack([r["stats"][0:E, 0] for r in results])   # [B, E]
    counts_f = np.stack([r["stats"][E : 2 * E, 0] for r in results])  # [B, E]
    expert_counts = np.rint(counts_f).astype(np.int32).sum(axis=0)
    # seq aux loss (exact reference formula, combined across cores)
    ce = counts_f * (E / (S * K))              # [B, E]
    mean_scores = score_sums / S               # [B, E]
    aux = np.float32((ce * mean_scores).sum(axis=1).mean() * ALPHA)
    return idx, wts, aux, expert_counts


def kernel(hidden_states: np.ndarray, weight: np.ndarray):
    hidden_states = np.asarray(hidden_states, dtype=np.float32)
    weight = np.ascontiguousarray(np.asarray(weight, dtype=np.float32))
    assert hidden_states.shape == (B, S, H) and weight.shape == (E, H)

    nc = _get_nc()
    in_maps = [
        {"x": np.ascontiguousarray(hidden_states[c]), "w": weight}
        for c in range(N_CORES)
    ]
    results = run_bass_kernel_spmd(nc, in_maps, list(range(N_CORES))).results
    return _combine(results)


# revision 4
# speedup vs baseline: 3.1097x; 3.1097x over previous
"""MoE gate (DeepSeek-style MoEGate) for Trainium2, 8 NeuronCores.

Problem: hidden_states [8, 4096, 1024] f32, router weight [64, 1024] f32.
  logits = tokens @ W.T ; scores = softmax(logits)
  top-8 indices/weights (renormalized), expert counts, seq-aux loss.

Sharding: data-parallel over the batch axis — core b handles batch row b
(4096 tokens). The router weight is replicated. Expert counts and the
aux-loss terms are computed per-core on device ([64] score sums + [64]
counts) and combined on the host (the "all-reduce" of the hint).

Per-core device pipeline (tokens on partitions, 128 per tile):
  DMA x [128,1024]-tiles -> PE transpose (128x128 blocks) -> PSUM -> copy
  to SBUF (x^T) -> PE matmul vs W^T chunks accumulating logits [128,64]
  -> max8/max_index (top-8 of 64 per partition) -> exp/sums on ACT/DVE
  -> renormalized top-8 weights; scores and top-8 mask are reduced over
  tokens with a ones-vector PE matmul into a persistent PSUM accumulator.
"""

import sys

if "/opt/trn_rl_repo" not in sys.path:
    sys.path.insert(0, "/opt/trn_rl_repo")

import numpy as np

import concourse.bass as bass
import concourse.mybir as mybir
import concourse.tile as tile
from concourse.bass import ds
from concourse.bass_utils import run_bass_kernel_spmd
from concourse.masks import make_identity

F32 = mybir.dt.float32
U32 = mybir.dt.uint32

B, S, H = 8, 4096, 1024
E, K = 64, 8
ALPHA = 0.01
N_CORES = 8

P = 128          # partitions / tokens per tile
HC = H // P      # 8 h-chunks of 128
SUP = 8          # token-tiles per super-tile
N_SUP = S // (SUP * P)   # 4 super-tiles per core


def _split_multi_waits(nc: bass.Bass, limit: int = 1):
    """This container's walrus build rejects any instruction carrying more
    than one semaphore wait (setupSyncWait: 'Too many sync wait commands'),
    while Tile freely attaches several. Post-pass: hoist excess waits onto
    single-wait NoOps inserted just before the instruction on the same
    engine (the engine's program-order queue preserves semantics)."""
    n = 0
    for f in nc.m.functions:
        for bb in f.blocks:
            insts = bb.instructions
            out = []
            changed = False
            for inst in insts:
                si = inst.sync_info
                if si is not None and len(si.on_wait) > limit:
                    waits = list(si.on_wait)
                    for w in waits[:-limit]:
                        nop = mybir.InstNoOp(
                            name=f"I-waitsplit-{n}",
                            sync_info=mybir.SyncInfo(on_update=[], on_wait=[w]),
                            bass_nofuse=True,
                            engine=inst.engine,
                        )
                        n += 1
                        out.append(nop)
                    si.on_wait = waits[-limit:]
                    changed = True
                out.append(inst)
            if changed:
                bb.instructions = out
    return n


def build_nc(repeat: int = 1) -> bass.Bass:
    """Build the per-core Bass program. repeat>1 replays the main token loop
    (for timing only; outputs are then garbage for the repeated passes)."""
    nc = bass.Bass()

    x_ext = nc.declare_dram_parameter("x", [S, H], F32, isOutput=False)
    w_ext = nc.declare_dram_parameter("w", [E, H], F32, isOutput=False)
    idx_ext = nc.declare_dram_parameter("idx", [S, K], U32, isOutput=True)
    wts_ext = nc.declare_dram_parameter("wts", [S, K], F32, isOutput=True)
    # stats rows 0:64  = sum over this core's tokens of softmax scores
    #       rows 64:128 = top-8 membership counts per expert
    stats_ext = nc.declare_dram_parameter("stats", [2 * E, 1], F32, isOutput=True)

    with tile.TileContext(nc) as tc:
        with tc.tile_pool(name="const", bufs=1) as const_pool:
            identity = const_pool.tile([P, P], F32)
            make_identity(nc, identity[:])
            ones = const_pool.tile([P, 1], F32)
            nc.vector.memset(ones[:], 1.0)

            # W^T chunks: wt[p, c, e] = W[e, c*128 + p]
            wt = const_pool.tile([P, HC, E], F32)
            w_sb = const_pool.tile([E, H], F32)
            nc.sync.dma_start(w_sb[:], w_ext[:])
            with tc.tile_pool(name="wt_psum", bufs=2, space="PSUM") as wt_psum_pool:
                for c in range(HC):
                    wt_ps = wt_psum_pool.tile([P, E], F32)
                    nc.tensor.transpose(
                        wt_ps[:], w_sb[:, ds(c * P, P)], identity[0:E, 0:E]
                    )
                    nc.vector.tensor_copy(wt[:, c, :], wt_ps[:])

            with (
                tc.tile_pool(name="x", bufs=3) as x_pool,
                tc.tile_pool(name="xT", bufs=3) as xT_pool,
                tc.tile_pool(name="lg", bufs=2) as lg_pool,
                tc.tile_pool(name="cb", bufs=2) as cb_pool,
                tc.tile_pool(name="sm", bufs=2) as sm_pool,
                tc.tile_pool(name="xT_psum", bufs=4, space="PSUM") as xT_psum_pool,
                tc.tile_pool(name="lg_psum", bufs=2, space="PSUM") as lg_psum_pool,
                tc.tile_pool(name="st_psum", bufs=1, space="PSUM") as st_psum_pool,
            ):
                stats_psum = st_psum_pool.tile([2 * E, 1], F32)

                for r in range(repeat):
                    for st in range(N_SUP):
                        t0 = st * SUP * P
                        x_t = x_pool.tile([P, SUP, H], F32)
                        for half in range(2):
                            src = x_ext[ds(t0 + half * 512, 512), :].rearrange(
                                "(s p) h -> p s h", p=P
                            )
                            nc.sync.dma_start(x_t[:, ds(half * 4, 4), :], src)

                        lg_psum = lg_psum_pool.tile([P, SUP, E], F32)
                        for j in range(SUP):
                            xT = xT_pool.tile([P, HC, P], F32)
                            for g in range(2):
                                xp = xT_psum_pool.tile([P, 4, P], F32)
                                for cc in range(4):
                                    c = g * 4 + cc
                                    nc.tensor.matmul(
                                        xp[:, cc, :],
                                        x_t[:, j, ds(c * P, P)],
                                        identity[:],
                                        start=True, stop=True,
                                    )
                                if g == 0:
                                    nc.vector.tensor_copy(
                                        xT[:, ds(g * 4, 4), :], xp[:]
                                    )
                                else:
                                    nc.scalar.copy(xT[:, ds(g * 4, 4), :], xp[:])
                            for c in range(HC):
                                nc.tensor.matmul(
                                    lg_psum[:, j, :],
                                    xT[:, c, :],
                                    wt[:, c, :],
                                    start=(c == 0),
                                    stop=(c == HC - 1),
                                )

                        lg = lg_pool.tile([P, SUP, E], F32)
                        nc.scalar.copy(lg[:], lg_psum[:])

                        v8 = sm_pool.tile([P, SUP, K], F32)
                        i8 = sm_pool.tile([P, SUP, K], U32)
                        for j in range(SUP):
                            nc.vector.max(out=v8[:, j, :], in_=lg[:, j, :])
                            nc.vector.max_index(
                                out=i8[:, j, :], in_max=v8[:, j, :],
                                in_values=lg[:, j, :],
                            )

                        # combined lhsT for the stats matmul:
                        # cols 0:64 = softmax scores, cols 64:128 = top-8 mask
                        cb = cb_pool.tile([P, SUP, 2 * E], F32)
                        nc.scalar.activation(
                            out=cb[:, :, 0:E], in_=lg[:],
                            func=mybir.ActivationFunctionType.Exp,
                        )
                        zs = sm_pool.tile([P, SUP, 1], F32)
                        nc.vector.tensor_reduce(
                            out=zs[:], in_=cb[:, :, 0:E],
                            axis=mybir.AxisListType.X, op=mybir.AluOpType.add,
                        )
                        zr = sm_pool.tile([P, SUP, 1], F32)
                        nc.vector.reciprocal(zr[:], zs[:])
                        nc.vector.tensor_tensor(
                            out=cb[:, :, 0:E], in0=cb[:, :, 0:E],
                            in1=zr[:].to_broadcast([P, SUP, E]),
                            op=mybir.AluOpType.mult,
                        )
                        nc.vector.tensor_tensor(
                            out=cb[:, :, E : 2 * E], in0=lg[:],
                            in1=v8[:, :, K - 1 : K].to_broadcast([P, SUP, E]),
                            op=mybir.AluOpType.is_ge,
                        )
                        for j in range(SUP):
                            nc.tensor.matmul(
                                stats_psum[:],
                                cb[:, j, :],
                                ones[:],
                                start=(r == 0 and st == 0 and j == 0),
                                stop=(r == repeat - 1 and st == N_SUP - 1
                                      and j == SUP - 1),
                                skip_group_check=True,
                            )

                        # renormalized top-8 weights: exp(v)/sum8(exp(v))
                        t8 = sm_pool.tile([P, SUP, K], F32)
                        nc.scalar.activation(
                            out=t8[:], in_=v8[:],
                            func=mybir.ActivationFunctionType.Exp,
                        )
                        s8 = sm_pool.tile([P, SUP, 1], F32)
                        nc.vector.tensor_reduce(
                            out=s8[:], in_=t8[:],
                            axis=mybir.AxisListType.X, op=mybir.AluOpType.add,
                        )
                        s8r = sm_pool.tile([P, SUP, 1], F32)
                        nc.vector.reciprocal(s8r[:], s8[:])
                        wout = sm_pool.tile([P, SUP, K], F32)
                        nc.vector.tensor_tensor(
                            out=wout[:], in0=t8[:],
                            in1=s8r[:].to_broadcast([P, SUP, K]),
                            op=mybir.AluOpType.mult,
                        )

                        dst_i = idx_ext[ds(t0, SUP * P), :].rearrange(
                            "(s p) k -> p s k", p=P
                        )
                        nc.scalar.dma_start(dst_i, i8[:])
                        dst_w = wts_ext[ds(t0, SUP * P), :].rearrange(
                            "(s p) k -> p s k", p=P
                        )
                        nc.scalar.dma_start(dst_w, wout[:])

                stats_sb = sm_pool.tile([2 * E, 1], F32)
                nc.vector.tensor_copy(stats_sb[:], stats_psum[:])
                nc.scalar.dma_start(stats_ext[:], stats_sb[:])

    nsplit = _split_multi_waits(nc)
    return nc


_NC_CACHE = {}


def _get_nc(repeat: int = 1) -> bass.Bass:
    if repeat not in _NC_CACHE:
        _NC_CACHE[repeat] = build_nc(repeat)
    return _NC_CACHE[repeat]


def _combine(results):
    idx = np.concatenate(
        [r["idx"].astype(np.int32) for r in results], axis=0
    )
    wts = np.concatenate([r["wts"] for r in results], axis=0)
    score_sums = np.stack([r["stats"][0:E, 0] for r in results])   # [B, E]
    counts_f = np.stack([r["stats"][E : 2 * E, 0] for r in results])  # [B, E]
    expert_counts = np.rint(counts_f).astype(np.int32).sum(axis=0)
    # seq aux loss (exact reference formula, combined across cores)
    ce = counts_f * (E / (S * K))              # [B, E]
    mean_scores = score_sums / S               # [B, E]
    aux = np.float32((ce * mean_scores).sum(axis=1).mean() * ALPHA)
    return idx, wts, aux, expert_counts


def kernel(hidden_states: np.ndarray, weight: np.ndarray):
    hidden_states = np.asarray(hidden_states, dtype=np.float32)
    weight = np.ascontiguousarray(np.asarray(weight, dtype=np.float32))
    assert hidden_states.shape == (B, S, H) and weight.shape == (E, H)

    nc = _get_nc()
    in_maps = [
        {"x": np.ascontiguousarray(hidden_states[c]), "w": weight}
        for c in range(N_CORES)
    ]
    results = run_bass_kernel_spmd(nc, in_maps, list(range(N_CORES))).results
    return _combine(results)
